# revision 3
# baseline (speedup 1.0000x reference)
"""CenterLoss kernel v10 for Trainium2 (raw Bass/Bacc), 8-core data-parallel.

loss = (1/B) * sum_i || x_i - centers[labels[i]] ||^2

v8 = v4a structure (fp8 staging, labels alone on the Sync HWDGE ring,
x via GpSimd SWDGE, 4 single-column indirect gathers, DVE subtract +
Scalar Square/accum per chunk) minus the final PE matmul + PSUM copy:
the 1/B scaling folds into the Square activation's input scale
(Square(dd/64) = dd^2/4096), the device emits the [128, CHUNKS] partial
sums, and the host all-reduces them (4096 adds across 8 cores).  This
removes ~0.6us of tail (matmul + PSUM->SBUF copy + cross-engine hops).

Notes discovered on the way here (do not regress):
  - multi-column indirect offset APs silently gather only column 0
  - custom-DVE ops (TENSOR_TENSOR_REDUCE etc.) stall SWDGE descriptor
    generation by ~2us per overlapped gather and can wedge the device
  - dma_gather's ucode is far slower than 4x indirect_dma_start here

Layout: chunk a holds samples a*128 + p on partition p.
  xt[p, a*F:(a+1)*F] = x[a*128 + p, :]
  lab_sb[p, a]       = labels[a*128 + p]
"""

from contextlib import ExitStack

import ml_dtypes
import numpy as np

import concourse.bacc as bacc
import concourse.bass as bass
from concourse import mybir
from concourse.bass_utils import run_bass_kernel_spmd

BATCH = 4096
NUM_CLASSES = 10000
FEAT_DIM = 512
N_CORES = 8
BPC = BATCH // N_CORES   # samples per core = 512
P = 128                  # SBUF partitions
CHUNKS = BPC // P        # 4 chunks of 128 samples per core

AF = mybir.AluOpType
DT, DT_NP = mybir.dt.float8e4, ml_dtypes.float8_e4m3
SQ_SCALE = 1.0 / 64.0    # Square(dd * 1/64) = dd^2 / 4096 = dd^2 / BATCH

_NC_CACHE = {}


def _build_bass():
    nc = bacc.Bacc(None, target_bir_lowering=False)

    x_in = nc.dram_tensor("x", [P, CHUNKS * FEAT_DIM], DT,
                          kind="ExternalInput")
    lab_in = nc.dram_tensor("labels", [P, CHUNKS], mybir.dt.int32,
                            kind="ExternalInput")
    cen_in = nc.dram_tensor("centers", [NUM_CLASSES, FEAT_DIM], DT,
                            kind="ExternalInput")
    out_t = nc.dram_tensor("out", [P, CHUNKS], mybir.dt.float32,
                           kind="ExternalOutput")

    with ExitStack() as ctx:
        ec = ctx.enter_context
        lab_sb = ec(nc.sbuf_tensor("lab_sb", [P, CHUNKS], mybir.dt.int32))
        xt = ec(nc.sbuf_tensor("xt", [P, CHUNKS * FEAT_DIM], DT))
        ct = ec(nc.sbuf_tensor("ct", [P, CHUNKS * FEAT_DIM], DT))
        dds = [ec(nc.sbuf_tensor(f"dd{a}", [P, FEAT_DIM], mybir.dt.bfloat16))
               for a in range(CHUNKS)]
        sqs = [ec(nc.sbuf_tensor(f"sq{a}", [P, FEAT_DIM], mybir.dt.bfloat16))
               for a in range(CHUNKS)]
        partials = ec(nc.sbuf_tensor("partials", [P, CHUNKS],
                                     mybir.dt.float32))
        s_lab = ec(nc.semaphore("s_lab"))
        s_x = ec(nc.semaphore("s_x"))
        s_cts = [ec(nc.semaphore(f"s_ct{a}")) for a in range(CHUNKS)]
        s_sub = ec(nc.semaphore("s_sub"))
        s_acc = ec(nc.semaphore("s_acc"))
        s_out = ec(nc.semaphore("s_out"))

        # ---- Sync: labels only (earliest possible completion sem) ----
        nc.sync.dma_start(out=lab_sb[:], in_=lab_in[:]).then_inc(s_lab, 16)

        # ---- Scalar: x load on the second HWDGE ring (keeps the x
        # descriptors out of the gathers' SWDGE ring, whose slow drain
        # otherwise stalls descriptor-gen on ring space) ----
        nc.scalar.dma_start(out=xt[:], in_=x_in[:]).then_inc(s_x, 16)

        # ---- GpSimd: 4 single-col gathers ----
        nc.gpsimd.wait_ge(s_lab, 16)
        for a in range(CHUNKS):
            nc.gpsimd.indirect_dma_start(
                out=ct[:, a * FEAT_DIM:(a + 1) * FEAT_DIM],
                out_offset=None,
                in_=cen_in[:],
                in_offset=bass.IndirectOffsetOnAxis(
                    ap=lab_sb[:, a:a + 1], axis=0),
            ).then_inc(s_cts[a], 16)

        # ---- Vector: per-chunk subtract ----
        nc.vector.wait_ge(s_x, 16)
        for a in range(CHUNKS):
            sl = slice(a * FEAT_DIM, (a + 1) * FEAT_DIM)
            nc.vector.wait_ge(s_cts[a], 16)
            nc.vector.tensor_tensor(
                out=dds[a][:], in0=xt[:, sl], in1=ct[:, sl],
                op=AF.subtract).then_inc(s_sub, 1)

        # ---- Scalar: per-chunk Square+accum, 1/B folded into the scale ----
        for a in range(CHUNKS):
            nc.scalar.wait_ge(s_sub, a + 1)
            nc.scalar.activation(
                out=sqs[a][:], in_=dds[a][:],
                func=mybir.ActivationFunctionType.Square,
                scale=SQ_SCALE,
                accum_out=partials[:, a:a + 1]).then_inc(s_acc, 1)

        # ---- Sync: output DMA of the partial sums (fire-and-forget) ----
        nc.sync.wait_ge(s_acc, CHUNKS)
        nc.sync.dma_start(out=out_t[:], in_=partials[:]).then_inc(s_out, 16)

    nc.compile()
    return nc


def get_nc():
    if "nc" not in _NC_CACHE:
        _NC_CACHE["nc"] = _build_bass()
    return _NC_CACHE["nc"]


def kernel(x, labels, centers, _run_kwargs=None):
    x = np.asarray(x)
    labels = np.asarray(labels)
    centers = np.asarray(centers)

    cen_dt = np.ascontiguousarray(centers).astype(DT_NP)
    lab32 = labels.astype(np.int32).reshape(N_CORES, CHUNKS, P)
    x_dt = x.astype(DT_NP).reshape(N_CORES, CHUNKS, P, FEAT_DIM)

    nc = get_nc()
    in_maps = []
    for c in range(N_CORES):
        xs = np.ascontiguousarray(
            x_dt[c].transpose(1, 0, 2)).reshape(P, CHUNKS * FEAT_DIM)
        ls = np.ascontiguousarray(lab32[c].T)
        in_maps.append({"x": xs, "labels": ls, "centers": cen_dt})

    kwargs = _run_kwargs or {}
    out = run_bass_kernel_spmd(nc, in_maps, core_ids=list(range(N_CORES)),
                               **kwargs)
    # all-reduce the per-core per-chunk partial sums (already / BATCH)
    total = np.float32(0.0)
    for r in out.results:
        total = total + r["out"].astype(np.float32).sum(dtype=np.float32)
    if kwargs:
        kernel.last_run = out
    return np.asarray(total, dtype=np.float32)
